# revision 1
# baseline (speedup 1.0000x reference)
"""Trainium2 Bass kernel for nn_Diffuser_78331613544465.

Math (per graph b of B=8, N=1024):
    A   = adj (mask is all-ones in the graded setup; general mask handled host-side)
    P   = A / max(rowsum(A), 1)
    out[i,j,:] = relu([I, P, P2, P4][i,j,:] @ w1 + b1) @ w2 + b2   (P2=P@P, P4=P2@P2)

Device strategy: data-parallel over B — one graph per NeuronCore (8 cores).
On-chip work happens in the TRANSPOSED domain (Q = P^T).  Because A is
symmetric, both P = D^-1 A (row scale) and Q = A D^-1 (col scale) come from
cheap elementwise scalings of A, and the power chain needs NO PE transposes:
    Q2 = P^T Q,  P2 = Q^T P,  Q4 = P2^T Q2      (matmul(lhsT=X, rhs=Y) = X^T Y)

The edge MLP processes 32 j's at a time with the PE split into 16 32x32
tiles (tile_position): the staged rhs holds, per 32-row group r, the four
channels [I, Q, Q2, Q4] x 8 j's of block jb=4G+r interleaved as partition
32r+8s+jj; tile (r,c) applies a constant selector weight picking j-pair
(2c,2c+1) x 16 hidden.  The identity channel replaces the separate diagonal
matmul, and bias b1 rides the relu.  Layer 2 is the K=128 block-diagonal
matmul, M=64 col-group-paired.  The [16j x 8o, i] PSUM result is evacuated
as-is (fp16) and the HOST un-transposes — this keeps every output-DMA
descriptor a full 2KB partition line instead of 128B chunks.

The channels are staged via a DRAM interleave tensor IL4[N, 4, N] (identity
channel uploaded by the host, Q/Q2/Q4 spilled per 128-row band during the
power chain) so each 32-j group loads with ONE 256KB DMA.

kernel(**inputs) takes FULL inputs, shards over 8 cores, returns FULL output.
"""

import os
import numpy as np

B, N, P = 8, 1024, 128
HID, HEADS, NSTACK = 16, 8, 4
NT = N // P          # 8 row-tiles
JBLK = 8             # j rows per MLP block
NJB = N // JBLK      # 128 j-blocks
IC = 512             # i-chunk (matmul free dim)
NIC = N // IC        # 2
NGRP = N // 32       # 32 j-groups of 32 j's (4 j-blocks)

_CACHE = {}
LAST_RESULTS = None


def _emit(nc, tc, ctx, mm_dt):
    from concourse import mybir

    f32 = mybir.dt.float32
    add = mybir.AluOpType.add
    amax = mybir.AluOpType.max
    mult = mybir.AluOpType.mult
    relu_fn = mybir.ActivationFunctionType.Relu

    adj = nc.declare_dram_parameter("adj", [N, N], f32, isOutput=False)
    w1sel_d = nc.declare_dram_parameter("w1sel", [P, P], mm_dt, isOutput=False)
    w2blk_d = nc.declare_dram_parameter("w2blk", [P, JBLK * HEADS], mm_dt, isOutput=False)
    b1rep_d = nc.declare_dram_parameter("b1rep", [P, 1], f32, isOutput=False)
    idn32_d = nc.declare_dram_parameter("idn32", [P, P], f32, isOutput=False)
    idnil_d = nc.declare_dram_parameter("idnil", [N, N], mm_dt, isOutput=False)
    # device-natural output: [jb-pair, (16j x 8o) partition, i] in fp16;
    # host transposes to [i, j, o] and casts to f32
    out = nc.declare_dram_parameter("out", [NJB // 2, P, N], mm_dt, isOutput=True)

    from contextlib import ExitStack

    small = ctx.enter_context(tc.tile_pool(name="small", bufs=1))
    big = ctx.enter_context(tc.tile_pool(name="big", bufs=1))
    spool = ctx.enter_context(tc.tile_pool(name="spool", bufs=3))
    rpool = ctx.enter_context(tc.tile_pool(name="rpool", bufs=8))
    ppool = ctx.enter_context(tc.tile_pool(name="ppool", bufs=3))
    dram = ctx.enter_context(tc.tile_pool(name="dram", bufs=1, space="DRAM"))
    ph14 = ExitStack()
    mm_ps = ph14.enter_context(tc.tile_pool(name="mm_ps", bufs=2, space="PSUM"))

    # persistent matrices, one [128, 1024] tile per 128-row band
    Af = [big.tile([P, N], mm_dt, name=f"Af{t}", tag=f"Af{t}") for t in range(NT)]
    Pf = [big.tile([P, N], mm_dt, name=f"Pf{t}", tag=f"Pf{t}") for t in range(NT)]
    Qf = [big.tile([P, N], mm_dt, name=f"Qf{t}", tag=f"Qf{t}") for t in range(NT)]
    Q2f = [big.tile([P, N], mm_dt, name=f"Q2f{t}", tag=f"Q2f{t}") for t in range(NT)]
    P2f = [big.tile([P, N], mm_dt, name=f"P2f{t}", tag=f"P2f{t}") for t in range(NT)]
    Q4f = [big.tile([P, N], mm_dt, name=f"Q4f{t}", tag=f"Q4f{t}") for t in range(NT)]
    invrep = big.tile([P, N], f32, tag="invrep")
    # DRAM channel-interleave [j, s, i]: s=0 identity (host), 1..3 = Q,Q2,Q4
    il4 = dram.tile([N, NSTACK, N], mm_dt, tag="il4")

    # ---- constants / weights (host-prepared; one DMA each) -----------------
    idn32 = small.tile([P, P], f32, tag="idn32")
    nc.gpsimd.dma_start(idn32[:], idn32_d[:])
    ones1 = small.tile([1, P], f32, tag="ones1")
    nc.vector.memset(ones1[:], 1.0)
    w1sel = small.tile([P, P], mm_dt, tag="w1sel")
    nc.gpsimd.dma_start(w1sel[:], w1sel_d[:])
    w2blk = small.tile([P, JBLK * HEADS], mm_dt, tag="w2blk")
    nc.gpsimd.dma_start(w2blk[:], w2blk_d[:])
    b1rep = small.tile([P, 1], f32, tag="b1rep")
    nc.gpsimd.dma_start(b1rep[:], b1rep_d[:])
    # identity channel of the interleave (DRAM -> DRAM, once)
    nc.sync.dma_start(il4[:, 0:1, :], idnil_d[:])

    # ---- phase 1: load adj (fp16 via DMA cast), deg -> invdeg, P ------------
    invcol = small.tile([P, NT], f32, tag="invcol")
    for t in range(NT):
        nc.gpsimd.dma_start(Af[t][:], adj[P * t:P * (t + 1), :])
        deg = small.tile([P, 1], f32, tag=f"deg{t}")
        nc.vector.tensor_reduce(
            deg[:], Af[t][:], axis=mybir.AxisListType.X, op=add,
        )
        degc = small.tile([P, 1], f32, tag=f"degc{t}")
        nc.vector.tensor_scalar_max(degc[:], deg[:], 1.0)
        nc.vector.reciprocal(invcol[:, t:t + 1], degc[:])
        # P = A * invdeg[row]  (per-partition scale on the scalar engine)
        nc.scalar.mul(Pf[t][:], Af[t][:], invcol[:, t:t + 1])

    # invrep[p, c] = invdeg(row c) for all p  (transpose + broadcast via PE)
    invrow = small.tile([1, N], f32, tag="invrow")
    for t in range(NT):
        ptp = mm_ps.tile([P, P], f32, tag="pt")
        nc.tensor.transpose(ptp[0:1, :], invcol[:, t:t + 1], idn32[:])
        nc.scalar.copy(invrow[0:1, P * t:P * (t + 1)], ptp[0:1, :])
    for half in range(2):
        pb = mm_ps.tile([P, IC], f32, tag="mm")
        for k in range(4):
            c = 4 * half + k
            nc.tensor.matmul(
                pb[:, P * k:P * (k + 1)], ones1[:], invrow[0:1, P * c:P * (c + 1)],
                start=True, stop=True,
            )
        nc.scalar.copy(invrep[:, IC * half:IC * (half + 1)], pb[:])

    # Q = A * invdeg[col]; spill each band into the interleave
    for t in range(NT):
        eng = nc.vector if t % 2 == 0 else nc.gpsimd
        eng.tensor_tensor(Qf[t][:], Af[t][:], invrep[:], op=mult)
        nc.sync.dma_start(il4[P * t:P * (t + 1), 1:2, :], Qf[t][:])

    # ---- power chain (no transposes; M3 := A D^-1 A is symmetric, so ONE
    # square yields both Q2 = M3 D^-1 (col scale) and P2 = D^-1 M3 (row
    # scale); then Q4 = Q2^2 = P2^T Q2) -------------------------------------
    for al in range(NT):
        mm = mm_ps.tile([P, N], f32, tag="mm")
        for be in range(NIC):
            for g in range(NT):
                nc.tensor.matmul(
                    mm[:, IC * be:IC * (be + 1)],
                    Af[g][:, P * al:P * (al + 1)],
                    Pf[g][:, IC * be:IC * (be + 1)],
                    start=(g == 0), stop=(g == NT - 1),
                )
        nc.vector.tensor_tensor(Q2f[al][:], mm[:], invrep[:], op=mult)
        nc.scalar.mul(P2f[al][:], mm[:], invcol[:, al:al + 1])
        nc.sync.dma_start(il4[P * al:P * (al + 1), 2:3, :], Q2f[al][:])
    for al in range(NT):
        mm = mm_ps.tile([P, N], f32, tag="mm")
        for be in range(NIC):
            for g in range(NT):
                nc.tensor.matmul(
                    mm[:, IC * be:IC * (be + 1)],
                    P2f[g][:, P * al:P * (al + 1)],
                    Q2f[g][:, IC * be:IC * (be + 1)],
                    start=(g == 0), stop=(g == NT - 1),
                )
        if al % 2 == 0:
            nc.scalar.copy(Q4f[al][:], mm[:])
        else:
            nc.vector.tensor_scalar_add(Q4f[al][:], mm[:], 0.0)
        nc.sync.dma_start(il4[P * al:P * (al + 1), 3:4, :], Q4f[al][:])
    ph14.close()  # free the mm PSUM banks for the MLP pools

    h_ps = ctx.enter_context(tc.tile_pool(name="h_ps", bufs=4, space="PSUM"))
    o_ps = ctx.enter_context(tc.tile_pool(name="o_ps", bufs=2, space="PSUM"))

    # ---- edge MLP: 32 j's per group, PE as 16 32x32 tiles -------------------
    for G in range(NGRP):
        stage = spool.tile([P, N], mm_dt, tag="S")
        # stage 4 channels x 8 j's per row group: partition 32r+8s+jj
        for r in range(4):
            j0 = 32 * G + 8 * r
            nc.gpsimd.dma_start(
                stage[32 * r:32 * (r + 1), :],
                il4[j0:j0 + 8, :, :].rearrange("jj s c -> s jj c"),
            )
        hs = {}
        for ic in range(NIC):
            for r in range(4):
                h = h_ps.tile([P, IC], f32, tag="H")
                hs[(r, ic)] = h
                for c in range(4):
                    nc.tensor.matmul(
                        h[32 * c:32 * (c + 1), :],
                        w1sel[32 * r:32 * (r + 1), 32 * c:32 * (c + 1)],
                        stage[32 * r:32 * (r + 1), IC * ic:IC * (ic + 1)],
                        start=True, stop=True, tile_position=(32 * r, 32 * c),
                    )
        rts = {}
        for ic in range(NIC):
            for r in range(4):
                rt = rpool.tile([P, IC], mm_dt, tag="R")
                rts[(r, ic)] = rt
                h = hs[(r, ic)]
                if (8 * G + 4 * ic + r) % 17 < 8:
                    nc.vector.tensor_scalar(rt[:], h[:], b1rep[:], 0.0, add, amax)
                else:
                    nc.scalar.activation(rt[:], h[:], relu_fn, bias=b1rep[:], scale=1.0)
        for half in range(2):
            po = o_ps.tile([P, N], f32, tag="O")
            for sub in range(2):
                for ic in range(NIC):
                    nc.tensor.matmul(
                        po[64 * sub:64 * (sub + 1), IC * ic:IC * (ic + 1)],
                        w2blk[:], rts[(2 * half + sub, ic)][:],
                        start=True, stop=True,
                    )
            ps = ppool.tile([P, N], mm_dt, tag="PS")
            if (2 * G + half) % 2 == 0:
                nc.vector.tensor_scalar_add(ps[:], po[:], 0.0)
            else:
                nc.scalar.copy(ps[:], po[:])
            nc.sync.dma_start(out[2 * G + half], ps[:])


def _build(mm_dtype_name="float16"):
    key = mm_dtype_name
    if key in _CACHE:
        return _CACHE[key]
    from contextlib import ExitStack
    import concourse.tile as tile
    from concourse import bacc, mybir

    nc = bacc.Bacc()
    with tile.TileContext(nc) as tc:
        with ExitStack() as ctx:
            _emit(nc, tc, ctx, getattr(mybir.dt, mm_dtype_name))
    nc.compile()
    _CACHE[key] = nc
    return nc


def _install_ntff_shim():
    """The agent image's antenv lacks axon_hooks; provide it and register the
    ctypes NTFF hook so run_bass_kernel_spmd(trace=True) can profile."""
    import sys
    import types

    if "antenv.axon_hooks" in sys.modules:
        return
    mod = types.ModuleType("antenv.axon_hooks")
    mod._hook = None
    mod.set_axon_ntff_profile_hook = lambda h: setattr(mod, "_hook", h)
    mod.get_axon_ntff_profile_hook = lambda: mod._hook
    sys.modules["antenv.axon_hooks"] = mod
    try:
        from trn_agent_boot.trn_boot import _ntff_profile_via_ctypes

        mod._hook = _ntff_profile_via_ctypes("/opt/axon/libaxon_pjrt.so")
    except Exception as e:  # degrade to no-trace
        print(f"ntff shim install failed: {e}")


def _host_tensors(w1, b1, w2, np_mm):
    # selector weights: tile (r,c) maps staged rows (s, jj) -> (jj', hid) of
    # j-pair (2c, 2c+1); identical for all four row groups r
    w1sel_np = np.zeros((P, P), np.float32)
    for r in range(4):
        for s in range(NSTACK):
            for c in range(4):
                for jj in range(2):
                    j = 2 * c + jj
                    w1sel_np[32 * r + JBLK * s + j,
                             32 * c + HID * jj:32 * c + HID * (jj + 1)] = w1[s]
    w2blk_np = np.zeros((P, JBLK * HEADS), np.float32)
    for j in range(JBLK):
        w2blk_np[HID * j:HID * (j + 1), HEADS * j:HEADS * (j + 1)] = w2
    return {
        "w1sel": w1sel_np.astype(np_mm),
        "w2blk": w2blk_np.astype(np_mm),
        "b1rep": np.ascontiguousarray(np.tile(b1, JBLK).astype(np.float32)[:, None]),
        "idn32": np.eye(P, dtype=np.float32),
        "idnil": np.eye(N, dtype=np_mm),
    }


def kernel(adj, mask, w1, b1, w2, b2):
    from concourse.bass_utils import run_bass_kernel_spmd

    global LAST_RESULTS
    adj = np.ascontiguousarray(np.asarray(adj, dtype=np.float32))
    mask = np.asarray(mask)
    w1 = np.ascontiguousarray(np.asarray(w1, dtype=np.float32))
    b1 = np.ascontiguousarray(np.asarray(b1, dtype=np.float32))
    w2 = np.ascontiguousarray(np.asarray(w2, dtype=np.float32))
    b2 = np.asarray(b2, dtype=np.float32)
    assert adj.shape == (B, N, N), adj.shape

    m = mask.astype(np.float32)
    general_mask = not np.all(m == 1.0)
    if general_mask:
        pair = m[:, :, None] * m[:, None, :]
        adj = np.ascontiguousarray(adj * pair)

    trace = bool(int(os.environ.get("KERNEL_TRACE", "0")))
    if trace:
        _install_ntff_shim()
    mmname = os.environ.get("KERNEL_MM_DT", "float16")
    nc = _build(mmname)

    from concourse import mybir

    np_mm = mybir.dt.np(getattr(mybir.dt, mmname))
    shared = _host_tensors(w1, b1, w2, np_mm)
    in_maps = [{"adj": adj[c], **shared} for c in range(B)]
    res = run_bass_kernel_spmd(nc, in_maps, list(range(B)), trace=trace)
    LAST_RESULTS = res

    outs = []
    for c in range(B):
        o2 = np.asarray(res.results[c]["out"])          # [64, 128, 1024] fp16
        o2 = o2.reshape(NJB // 2, 2, JBLK, HEADS, N)    # [pi, sub, j', o, i]
        o2 = np.transpose(o2, (4, 0, 1, 2, 3))          # [i, pi, sub, j', o]
        outs.append(o2.reshape(N, N, HEADS).astype(np.float32))
    outp = np.stack(outs, axis=0)

    if np.any(b2 != 0.0):
        outp = outp + b2
    if general_mask:
        outp = outp * pair[..., None]
    return np.ascontiguousarray(outp.astype(np.float32))



# revision 3
# speedup vs baseline: 1.0451x; 1.0451x over previous
"""Trainium2 Bass kernel for nn_Diffuser_78331613544465.

Math (per graph b of B=8, N=1024):
    A   = adj (mask is all-ones in the graded setup; general mask handled host-side)
    P   = A / max(rowsum(A), 1)
    out[i,j,:] = relu([I, P, P2, P4][i,j,:] @ w1 + b1) @ w2 + b2   (P2=P@P, P4=P2@P2)

Device strategy: data-parallel over B — one graph per NeuronCore (8 cores).
On-chip work happens in the TRANSPOSED domain (Q = P^T).  Because A is
symmetric, both P = D^-1 A (row scale) and Q = A D^-1 (col scale) come from
cheap elementwise scalings of A, and the power chain needs NO PE transposes:
    Q2 = P^T Q,  P2 = Q^T P,  Q4 = P2^T Q2      (matmul(lhsT=X, rhs=Y) = X^T Y)

Edge MLP layer 1 runs as 4 CONCURRENT row-tiled matmuls (tile_position=(32r,0),
K=32 = 4 stacks x 8 j's, M=128 = 8 j x 16 hid) per 512-i half — 8 matmuls per
32-j group instead of 32.  PSUM h tiles are [128,1024] (two banks, two j-blocks)
so the relu+bias evacuation runs at FD=1024, alternating Vector/Scalar (the only
two engines with PSUM access).  Layer 2 is K=128 block-diagonal with the two
j-blocks of a pair issued as concurrent column tiles (tile_position=(0,0)/(0,64)
against a duplicated weight [w2blk|w2blk]).

Channels are staged through a DRAM interleave il4[jb, s, jj, i] (jb = j-block,
s = 0:I (host-uploaded), 1:Q, 2:Q2, 3:Q4) laid out so that every group stage
load is one CONTIGUOUS 64KB DMA per j-block and every band spill is a simple
2-level affine pattern — no expensive gather descriptors.

The MLP groups are EMITTED INTERLEAVED into the second power-chain square with
a one-band lag, so the Vector/Scalar evacuation work (the MLP bottleneck)
overlaps the PE-bound power chain.

The [16j x 8o, i] PSUM result is evacuated as fp16 and the HOST un-transposes
— this keeps every output-DMA descriptor a full 2KB partition line.

kernel(**inputs) takes FULL inputs, shards over 8 cores, returns FULL output.
"""

import os
import numpy as np

B, N, P = 8, 1024, 128
HID, HEADS, NSTACK = 16, 8, 4
NT = N // P          # 8 row-tiles
JBLK = 8             # j rows per MLP block
NJB = N // JBLK      # 128 j-blocks
IC = 512             # i-chunk (matmul free dim)
NIC = N // IC        # 2
NGRP = N // 32       # 32 j-groups of 32 j's (4 j-blocks)

_CACHE = {}
LAST_RESULTS = None


def _emit(nc, tc, ctx, mm_dt):
    from concourse import mybir

    f32 = mybir.dt.float32
    add = mybir.AluOpType.add
    amax = mybir.AluOpType.max
    mult = mybir.AluOpType.mult
    relu_fn = mybir.ActivationFunctionType.Relu

    adj = nc.declare_dram_parameter("adj", [N, N], f32, isOutput=False)
    w1row_d = nc.declare_dram_parameter("w1row", [P, P], mm_dt, isOutput=False)
    w2pair_d = nc.declare_dram_parameter("w2pair", [P, P], mm_dt, isOutput=False)
    b1rep_d = nc.declare_dram_parameter("b1rep", [P, 1], f32, isOutput=False)
    idn32_d = nc.declare_dram_parameter("idn32", [P, P], f32, isOutput=False)
    idnil_d = nc.declare_dram_parameter("idnil", [NJB, JBLK, N], mm_dt, isOutput=False)
    # device-natural output: [jb-pair, (16j x 8o) partition, i] in fp16;
    # host transposes to [i, j, o] and casts to f32
    out = nc.declare_dram_parameter("out", [NJB // 2, P, N], mm_dt, isOutput=True)

    small = ctx.enter_context(tc.tile_pool(name="small", bufs=1))
    big = ctx.enter_context(tc.tile_pool(name="big", bufs=1))
    spool = ctx.enter_context(tc.tile_pool(name="spool", bufs=6))
    rpool = ctx.enter_context(tc.tile_pool(name="rpool", bufs=4))
    ppool = ctx.enter_context(tc.tile_pool(name="ppool", bufs=5))
    dram = ctx.enter_context(tc.tile_pool(name="dram", bufs=1, space="DRAM"))
    # PSUM budget (8 banks): mm 2x[128,512]=2, h 2x[128,1024]=4, o 2x[128,512]=2
    mm_ps = ctx.enter_context(tc.tile_pool(name="mm_ps", bufs=2, space="PSUM"))
    h_ps = ctx.enter_context(tc.tile_pool(name="h_ps", bufs=2, space="PSUM"))
    o_ps = ctx.enter_context(tc.tile_pool(name="o_ps", bufs=2, space="PSUM"))

    # persistent matrices, one [128, 1024] tile per 128-row band
    Af = [big.tile([P, N], mm_dt, name=f"Af{t}", tag=f"Af{t}") for t in range(NT)]
    Pf = [big.tile([P, N], mm_dt, name=f"Pf{t}", tag=f"Pf{t}") for t in range(NT)]
    Qf = [big.tile([P, N], mm_dt, name=f"Qf{t}", tag=f"Qf{t}") for t in range(NT)]
    Q2f = [big.tile([P, N], mm_dt, name=f"Q2f{t}", tag=f"Q2f{t}") for t in range(NT)]
    P2f = [big.tile([P, N], mm_dt, name=f"P2f{t}", tag=f"P2f{t}") for t in range(NT)]
    Q4f = [big.tile([P, N], mm_dt, name=f"Q4f{t}", tag=f"Q4f{t}") for t in range(NT)]
    invrep = big.tile([P, N], f32, tag="invrep")
    # DRAM channel-interleave [jb, s, jj, i]: s=0 identity (host), 1..3 = Q,Q2,Q4
    il4 = dram.tile([NJB, NSTACK, JBLK, N], mm_dt, tag="il4")

    # ---- constants / weights (host-prepared; one DMA each) -----------------
    idn32 = small.tile([P, P], f32, tag="idn32")
    nc.gpsimd.dma_start(idn32[:], idn32_d[:])
    ones1 = small.tile([1, P], f32, tag="ones1")
    nc.vector.memset(ones1[:], 1.0)
    w1row = small.tile([P, P], mm_dt, tag="w1row")
    nc.gpsimd.dma_start(w1row[:], w1row_d[:])
    w2pair = small.tile([P, P], mm_dt, tag="w2pair")
    nc.gpsimd.dma_start(w2pair[:], w2pair_d[:])
    b1rep = small.tile([P, 1], f32, tag="b1rep")
    nc.gpsimd.dma_start(b1rep[:], b1rep_d[:])
    # identity channel of the interleave (DRAM -> DRAM, once)
    nc.sync.dma_start(il4[:, 0:1, :, :], idnil_d[:])

    # ---- phase 1: load adj (fp16 via DMA cast, 4 queue-spread chunks per
    # band), deg -> invdeg, P ------------------------------------------------
    invcol = small.tile([P, NT], f32, tag="invcol")
    for t in range(NT):
        for q in range(4):
            # fp32 -> fp16 casting DMA: gpsimd (SWDGE) only
            nc.gpsimd.dma_start(
                Af[t][:, 256 * q:256 * (q + 1)],
                adj[P * t:P * (t + 1), 256 * q:256 * (q + 1)],
            )
        deg = small.tile([P, 1], f32, tag=f"deg{t}")
        nc.vector.tensor_reduce(
            deg[:], Af[t][:], axis=mybir.AxisListType.X, op=add,
        )
        degc = small.tile([P, 1], f32, tag=f"degc{t}")
        nc.vector.tensor_scalar_max(degc[:], deg[:], 1.0)
        nc.vector.reciprocal(invcol[:, t:t + 1], degc[:])
        # P = A * invdeg[row]  (per-partition scale on the scalar engine)
        nc.scalar.mul(Pf[t][:], Af[t][:], invcol[:, t:t + 1])

    # invrep[p, c] = invdeg(row c) for all p  (transpose + broadcast via PE)
    invrow = small.tile([1, N], f32, tag="invrow")
    for t in range(NT):
        ptp = mm_ps.tile([P, IC], f32, tag="mm")
        nc.tensor.transpose(ptp[0:1, 0:P], invcol[:, t:t + 1], idn32[:])
        nc.scalar.copy(invrow[0:1, P * t:P * (t + 1)], ptp[0:1, 0:P])
    for half in range(2):
        pb = mm_ps.tile([P, IC], f32, tag="mm")
        for k in range(4):
            c = 4 * half + k
            nc.tensor.matmul(
                pb[:, P * k:P * (k + 1)], ones1[:], invrow[0:1, P * c:P * (c + 1)],
                start=True, stop=True,
            )
        nc.scalar.copy(invrep[:, IC * half:IC * (half + 1)], pb[:])

    # Q = A * invdeg[col]; spill each band into the interleave
    for t in range(NT):
        eng = nc.vector if t % 2 == 0 else nc.gpsimd
        eng.tensor_tensor(Qf[t][:], Af[t][:], invrep[:], op=mult)
        nc.sync.dma_start(il4[16 * t:16 * (t + 1), 1:2, :, :], Qf[t][:])

    # ---- power chain (no transposes; M3 := A D^-1 A is symmetric, so ONE
    # square yields both Q2 = M3 D^-1 (col scale) and P2 = D^-1 M3 (row
    # scale); then Q4 = Q2^2 = P2^T Q2) -------------------------------------
    for al in range(NT):
        for be in range(NIC):
            mm = mm_ps.tile([P, IC], f32, tag="mm")
            for g in range(NT):
                nc.tensor.matmul(
                    mm[:],
                    Af[g][:, P * al:P * (al + 1)],
                    Pf[g][:, IC * be:IC * (be + 1)],
                    start=(g == 0), stop=(g == NT - 1),
                )
            nc.vector.tensor_tensor(
                Q2f[al][:, IC * be:IC * (be + 1)], mm[:],
                invrep[:, IC * be:IC * (be + 1)], op=mult,
            )
            nc.scalar.mul(
                P2f[al][:, IC * be:IC * (be + 1)], mm[:], invcol[:, al:al + 1],
            )
        nc.sync.dma_start(il4[16 * al:16 * (al + 1), 2:3, :, :], Q2f[al][:])

    # ---- edge MLP group emitter (32 j's = 4 j-blocks per group) ------------
    def emit_group(G):
        stage = spool.tile([P, N], mm_dt, tag="S")
        # one contiguous 64KB DMA per j-block: partitions (r, s, jj)
        for r in range(4):
            eng = nc.scalar if r % 2 == 0 else nc.sync
            eng.dma_start(
                stage[32 * r:32 * (r + 1), :],
                il4[4 * G + r:4 * G + r + 1, :, :, :],
            )
        rts = {}
        for ic in range(NIC):
            # 4 concurrent row-tiled matmuls: K=32, M=128, N=512.
            # hA = j-blocks (4G+0, 4G+1), hB = (4G+2, 4G+3); one bank per MM.
            hA = h_ps.tile([P, N], f32, tag="H")
            hB = h_ps.tile([P, N], f32, tag="H")
            for r in range(4):
                dst = hA if r < 2 else hB
                nc.tensor.matmul(
                    dst[:, IC * (r % 2):IC * (r % 2 + 1)],
                    w1row[32 * r:32 * (r + 1), :],
                    stage[32 * r:32 * (r + 1), IC * ic:IC * (ic + 1)],
                    start=True, stop=True, tile_position=(32 * r, 0),
                )
            # relu+bias evacuation at FD=1024, split across the only two
            # engines with PSUM access
            rtA = rpool.tile([P, N], mm_dt, tag="R")
            rtB = rpool.tile([P, N], mm_dt, tag="R")
            nc.vector.tensor_scalar(rtA[:], hA[:], b1rep[:], 0.0, add, amax)
            nc.scalar.activation(rtB[:], hB[:], relu_fn, bias=b1rep[:], scale=1.0)
            rts[ic] = (rtA, rtB)
        for rp in range(2):
            psout = ppool.tile([P, N], mm_dt, tag="PS")
            for ic in range(NIC):
                po = o_ps.tile([P, IC], f32, tag="O")
                src = rts[ic][rp]
                # two concurrent column tiles: j-blocks 4G+2rp / 4G+2rp+1
                nc.tensor.matmul(
                    po[0:64, :], w2pair[:, 0:64], src[:, 0:IC],
                    start=True, stop=True, tile_position=(0, 0),
                )
                nc.tensor.matmul(
                    po[64:128, :], w2pair[:, 64:128], src[:, IC:N],
                    start=True, stop=True, tile_position=(0, 64),
                )
                if (G + 2 * rp + ic) % 2 == 0:
                    nc.vector.tensor_scalar_add(
                        psout[:, IC * ic:IC * (ic + 1)], po[:], 0.0,
                    )
                else:
                    nc.scalar.copy(psout[:, IC * ic:IC * (ic + 1)], po[:])
            nc.sync.dma_start(out[2 * G + rp], psout[:])

    # ---- second square, MLP groups interleaved with one-band lag -----------
    for al in range(NT):
        for be in range(NIC):
            mm = mm_ps.tile([P, IC], f32, tag="mm")
            for g in range(NT):
                nc.tensor.matmul(
                    mm[:],
                    P2f[g][:, P * al:P * (al + 1)],
                    Q2f[g][:, IC * be:IC * (be + 1)],
                    start=(g == 0), stop=(g == NT - 1),
                )
            if be == 0:
                nc.vector.tensor_scalar_add(Q4f[al][:, 0:IC], mm[:], 0.0)
            else:
                nc.scalar.copy(Q4f[al][:, IC:N], mm[:])
        nc.sync.dma_start(il4[16 * al:16 * (al + 1), 3:4, :, :], Q4f[al][:])
        if al >= 1:
            for G in range(4 * (al - 1), 4 * al):
                emit_group(G)
    for G in range(4 * (NT - 1), 4 * NT):
        emit_group(G)


def _build(mm_dtype_name="float16"):
    key = mm_dtype_name
    if key in _CACHE:
        return _CACHE[key]
    from contextlib import ExitStack
    import concourse.tile as tile
    from concourse import bacc, mybir

    nc = bacc.Bacc()
    with tile.TileContext(nc) as tc:
        with ExitStack() as ctx:
            _emit(nc, tc, ctx, getattr(mybir.dt, mm_dtype_name))
    nc.compile()
    _CACHE[key] = nc
    return nc


def _install_ntff_shim():
    """The agent image's antenv lacks axon_hooks; provide it and register the
    ctypes NTFF hook so run_bass_kernel_spmd(trace=True) can profile."""
    import sys
    import types

    if "antenv.axon_hooks" in sys.modules:
        return
    mod = types.ModuleType("antenv.axon_hooks")
    mod._hook = None
    mod.set_axon_ntff_profile_hook = lambda h: setattr(mod, "_hook", h)
    mod.get_axon_ntff_profile_hook = lambda: mod._hook
    sys.modules["antenv.axon_hooks"] = mod
    try:
        from trn_agent_boot.trn_boot import _ntff_profile_via_ctypes

        mod._hook = _ntff_profile_via_ctypes("/opt/axon/libaxon_pjrt.so")
    except Exception as e:  # degrade to no-trace
        print(f"ntff shim install failed: {e}")


def _host_tensors(w1, b1, w2, np_mm):
    # L1 row-tile weights: strip r rows (8s+jj) -> cols (16jj+h); identical
    # for all four row strips r
    w1row_np = np.zeros((P, P), np.float32)
    for r in range(4):
        for s in range(NSTACK):
            for jj in range(JBLK):
                w1row_np[32 * r + JBLK * s + jj,
                         HID * jj:HID * (jj + 1)] = w1[s]
    # L2 weights: block-diagonal w2 per j, duplicated for the two column tiles
    w2pair_np = np.zeros((P, P), np.float32)
    for j in range(JBLK):
        w2pair_np[HID * j:HID * (j + 1), HEADS * j:HEADS * (j + 1)] = w2
        w2pair_np[HID * j:HID * (j + 1), 64 + HEADS * j:64 + HEADS * (j + 1)] = w2
    idnil_np = np.eye(N, dtype=np_mm).reshape(NJB, JBLK, N)
    return {
        "w1row": w1row_np.astype(np_mm),
        "w2pair": w2pair_np.astype(np_mm),
        "b1rep": np.ascontiguousarray(np.tile(b1, JBLK).astype(np.float32)[:, None]),
        "idn32": np.eye(P, dtype=np.float32),
        "idnil": idnil_np,
    }


def kernel(adj, mask, w1, b1, w2, b2):
    from concourse.bass_utils import run_bass_kernel_spmd

    global LAST_RESULTS
    adj = np.ascontiguousarray(np.asarray(adj, dtype=np.float32))
    mask = np.asarray(mask)
    w1 = np.ascontiguousarray(np.asarray(w1, dtype=np.float32))
    b1 = np.ascontiguousarray(np.asarray(b1, dtype=np.float32))
    w2 = np.ascontiguousarray(np.asarray(w2, dtype=np.float32))
    b2 = np.asarray(b2, dtype=np.float32)
    assert adj.shape == (B, N, N), adj.shape

    m = mask.astype(np.float32)
    general_mask = not np.all(m == 1.0)
    if general_mask:
        pair = m[:, :, None] * m[:, None, :]
        adj = np.ascontiguousarray(adj * pair)

    trace = bool(int(os.environ.get("KERNEL_TRACE", "0")))
    if trace:
        _install_ntff_shim()
    mmname = os.environ.get("KERNEL_MM_DT", "float16")
    nc = _build(mmname)

    from concourse import mybir

    np_mm = mybir.dt.np(getattr(mybir.dt, mmname))
    shared = _host_tensors(w1, b1, w2, np_mm)
    in_maps = [{"adj": adj[c], **shared} for c in range(B)]
    res = run_bass_kernel_spmd(nc, in_maps, list(range(B)), trace=trace)
    LAST_RESULTS = res

    outs = []
    for c in range(B):
        o2 = np.asarray(res.results[c]["out"])          # [64, 128, 1024] fp16
        o2 = o2.reshape(NJB // 2, 2, JBLK, HEADS, N)    # [pi, sub, j', o, i]
        o2 = np.transpose(o2, (4, 0, 1, 2, 3))          # [i, pi, sub, j', o]
        outs.append(o2.reshape(N, N, HEADS).astype(np.float32))
    outp = np.stack(outs, axis=0)

    if np.any(b2 != 0.0):
        outp = outp + b2
    if general_mask:
        outp = outp * pair[..., None]
    return np.ascontiguousarray(outp.astype(np.float32))
